# revision 1
# baseline (speedup 1.0000x reference)
"""AFNO block (nn_Block_32109175505281) on 8 Trainium2 NeuronCores.

Decomposition (3 SPMD launches, host reshard between):
  P1 token-sharded: LN1 (no affine; g folded into einsum weights, b via DC fix)
     + PE-transpose -> channel-major h1cm[b, blk, c96, h128, w256]
  P2 unit-sharded (16 units (b,blk), 2/core): matmul-DFT rfft2, 2-layer
     block-diagonal complex MLP (relu, softshrink folded into relu bias),
     matmul-DFT irfft2 -> h2cm (channel-major)
  P3 token-sharded: LN2 (stats via ones-matmul in transposed layout),
     MLP 768->3072->3072->768 (exact GELU), +residual -> out token-major

All big matmuls run as float32r (full-rate fp32 on PE for moving dim>=256).
"""
import sys
import numpy as np

sys.path.insert(0, '/opt/trn_rl_repo')

import concourse.bacc as bacc
import concourse.tile as tile
import concourse.mybir as mybir
from concourse.bass_utils import run_bass_kernel_spmd
from concourse.masks import make_identity

F32 = mybir.dt.float32
F32R = mybir.dt.float32r
AF = mybir.ActivationFunctionType

H, W, NB, BS, D = 128, 256, 8, 96, 768
Wf = W // 2 + 1        # 129
HW = H * W             # 32768
HID = 4 * D            # 3072
LAM = 0.01
EPS = 1e-5
SQHW = float(np.sqrt(H * W))
NCORES = 8
TPC = 2 * HW // NCORES  # tokens per core = 8192
HSLAB = H // 4          # 32 h-rows per core (4 cores per batch)
P = H * Wf              # 16512 frequency points per unit
TG = 512                # phase-3 token group

_programs = {}


# ---------------------------------------------------------------- matrices
def build_mats():
    f64 = np.float64
    h = np.arange(H, dtype=f64)
    u = np.arange(H, dtype=f64)
    w = np.arange(W, dtype=f64)
    v = np.arange(Wf, dtype=f64)
    th = 2 * np.pi * np.outer(h, u) / H
    Ecat = np.concatenate([np.cos(th), -np.sin(th)], axis=1) / SQHW  # [128,256]
    tw = 2 * np.pi * np.outer(w, v) / W
    Fr, Fs = np.cos(tw), np.sin(tw)
    Fcat1 = np.concatenate([Fr, -Fs], axis=1)  # [256,258]
    Fcat2 = np.concatenate([Fs, Fr], axis=1)
    thi = 2 * np.pi * np.outer(u, h) / H
    CS = np.concatenate([np.cos(thi), np.sin(thi)], axis=1) / SQHW   # [128,256]
    mu = np.ones(Wf); mu[1:W // 2] = 2.0
    twi = 2 * np.pi * np.outer(v, w) / W
    cw_full = mu[:, None] * np.cos(twi)
    sw_full = -mu[:, None] * np.sin(twi)
    c = lambda a: np.ascontiguousarray(a, dtype=np.float32)
    return dict(Ecat=c(Ecat),
                F1=c(Fcat1.reshape(2, 128, 258).transpose(1, 0, 2)),  # [128,2,258]
                F2=c(Fcat2.reshape(2, 128, 258).transpose(1, 0, 2)),
                CS=c(CS), cw=c(cw_full[:128]), sw=c(sw_full[:128]),
                cwn=c(cw_full[128:129]))


# ---------------------------------------------------------------- phase 1
def build_phase1():
    nc = bacc.Bacc(None, target_bir_lowering=False)
    xs = nc.dram_tensor("xs", [TPC, D], F32, kind="ExternalInput")
    o = nc.dram_tensor("o", [NB, BS, HSLAB, W], F32, kind="ExternalOutput")

    with tile.TileContext(nc) as tc:
        with tc.tile_pool(name="single", bufs=1) as single, \
             tc.tile_pool(name="xt", bufs=3) as xtp, \
             tc.tile_pool(name="st", bufs=3) as stp, \
             tc.tile_pool(name="ot", bufs=6) as otp, \
             tc.tile_pool(name="ps", bufs=6, space="PSUM") as psp:
            ident = single.tile([128, 128], F32)
            make_identity(nc, ident)
            epst = single.tile([128, 1], F32)
            nc.vector.memset(epst, EPS)

            ntiles = TPC // 128  # 64
            for t in range(ntiles):
                hl, wc = t // 2, t % 2
                xt = xtp.tile([128, D], F32)
                nc.sync.dma_start(xt, xs[t * 128:(t + 1) * 128, :])
                st = stp.tile([128, 3, 6], F32)
                for sg in range(3):
                    nc.vector.bn_stats(st[:, sg, :],
                                       xt[:, sg * 256:(sg + 1) * 256])
                mv = stp.tile([128, 2], F32)
                nc.vector.bn_aggr(mv, st)
                rstd = stp.tile([128, 1], F32)
                nc.scalar.activation(rstd, mv[:, 1:2], AF.Sqrt,
                                     bias=epst[:, 0:1], scale=1.0)
                nc.vector.reciprocal(rstd, rstd)
                nc.vector.tensor_scalar(out=xt, in0=xt,
                                        scalar1=mv[:, 0:1], scalar2=rstd,
                                        op0=mybir.AluOpType.subtract,
                                        op1=mybir.AluOpType.mult)
                for blk in range(NB):
                    pt = psp.tile([96, 128], F32, name="pt")
                    nc.tensor.transpose(pt, xt[:, blk * BS:(blk + 1) * BS],
                                        ident)
                    ot = otp.tile([96, 128], F32)
                    if blk % 2 == 0:
                        nc.vector.tensor_copy(ot, pt)
                    else:
                        nc.scalar.copy(ot, pt)
                    nc.sync.dma_start(
                        o[blk, :, hl, wc * 128:(wc + 1) * 128], ot)
    nc.compile()
    return nc


# ---------------------------------------------------------------- phase 2
def build_phase2():
    nc = bacc.Bacc(None, target_bir_lowering=False)
    data = nc.dram_tensor("data", [2, BS, H, W], F32, kind="ExternalInput")
    # dft mats
    ecat = nc.dram_tensor("ecat", [128, 256], F32, kind="ExternalInput")
    f1 = nc.dram_tensor("f1", [128, 2, 258], F32, kind="ExternalInput")
    f2 = nc.dram_tensor("f2", [128, 2, 258], F32, kind="ExternalInput")
    cs = nc.dram_tensor("cs", [128, 256], F32, kind="ExternalInput")
    cw = nc.dram_tensor("cw", [128, 256], F32, kind="ExternalInput")
    sw = nc.dram_tensor("sw", [128, 256], F32, kind="ExternalInput")
    cwn = nc.dram_tensor("cwn", [1, 256], F32, kind="ExternalInput")
    # per-unit einsum weights/biases (g folded; wXn = negated)
    wts = {}
    for name in ["w1r", "w1i", "w1in", "w2r", "w2i", "w2in"]:
        wts[name] = nc.dram_tensor(name, [2, 96, 96], F32, kind="ExternalInput")
    bias = {}
    for name in ["b1r", "b1i", "b2r", "b2i", "bdc"]:
        bias[name] = nc.dram_tensor(name, [2, 96, 1], F32, kind="ExternalInput")
    y = nc.dram_tensor("y", [2, BS, H, W], F32, kind="ExternalOutput")

    CH = [(s, min(s + 512, P)) for s in range(0, P, 512)]  # 33 chunks

    with tile.TileContext(nc) as tc:
        with tc.tile_pool(name="single", bufs=1) as single, \
             tc.tile_pool(name="uw", bufs=2) as uwp, \
             tc.tile_pool(name="din", bufs=3) as dinp, \
             tc.tile_pool(name="zt", bufs=4) as ztp, \
             tc.tile_pool(name="xt", bufs=3) as xtp, \
             tc.tile_pool(name="ex", bufs=4) as exp_, \
             tc.tile_pool(name="r12", bufs=4) as r12p, \
             tc.tile_pool(name="inv", bufs=4) as invp, \
             tc.tile_pool(name="yt", bufs=4) as ytp, \
             tc.tile_pool(name="psa", bufs=4, space="PSUM") as psa, \
             tc.tile_pool(name="pse", bufs=4, space="PSUM") as pse, \
             tc.tile_pool(name="dram", bufs=2, space="DRAM") as dram:
            # resident mats as f32r (gpsimd dma casts)
            ecat_t = single.tile([128, 256], F32R)
            nc.gpsimd.dma_start(ecat_t, ecat[:, :])
            f1_t = single.tile([128, 2, 258], F32R)
            nc.gpsimd.dma_start(f1_t, f1[:, :, :])
            f2_t = single.tile([128, 2, 258], F32R)
            nc.gpsimd.dma_start(f2_t, f2[:, :, :])
            cs_t = single.tile([128, 256], F32R)
            nc.gpsimd.dma_start(cs_t, cs[:, :])
            cw_t = single.tile([128, 256], F32R)
            nc.gpsimd.dma_start(cw_t, cw[:, :])
            sw_t = single.tile([128, 256], F32R)
            nc.gpsimd.dma_start(sw_t, sw[:, :])
            cwn_t = single.tile([1, 256], F32R)
            nc.gpsimd.dma_start(cwn_t, cwn[:, :])

            for un in range(2):
                # unit weights
                wt = {}
                for name in ["w1r", "w1i", "w1in", "w2r", "w2i", "w2in"]:
                    wt[name] = uwp.tile([96, 96], F32R, name=name)
                    nc.gpsimd.dma_start(wt[name], wts[name][un, :, :])
                bt = {}
                for name in ["b1r", "b1i", "b2r", "b2i"]:
                    bt[name] = uwp.tile([96, 1], F32, name=name)
                    nc.sync.dma_start(bt[name], bias[name][un, :, :])
                bdc_t = uwp.tile([96, 1], F32R, name="bdc")
                nc.gpsimd.dma_start(bdc_t, bias["bdc"][un, :, :])

                str_xr = dram.tile([BS, P], F32, name="sxr")
                str_xi = dram.tile([BS, P], F32, name="sxi")
                str_r2 = dram.tile([BS, P], F32, name="sr2")
                str_i2 = dram.tile([BS, P], F32, name="si2")

                # ---- forward DFT per channel
                for c in range(BS):
                    din = dinp.tile([128, 256], F32R)
                    nc.gpsimd.dma_start(din, data[un, c, :, :])
                    z0 = psa.tile([128, 256], F32, name="a")
                    z1 = psa.tile([128, 256], F32, name="a")
                    nc.tensor.matmul(z0, din[:, 0:128], ecat_t,
                                     start=True, stop=True)
                    nc.tensor.matmul(z1, din[:, 128:256], ecat_t,
                                     start=True, stop=True)
                    zs0 = ztp.tile([128, 256], F32R, name="zs")
                    zs1 = ztp.tile([128, 256], F32R, name="zs")
                    nc.vector.tensor_copy(zs0, z0)
                    nc.scalar.copy(zs1, z1)
                    px = psa.tile([128, 258], F32, name="a")
                    nc.tensor.matmul(px, zs0[:, 0:128], f1_t[:, 0, :],
                                     start=True, stop=False)
                    nc.tensor.matmul(px, zs0[:, 128:256], f2_t[:, 0, :],
                                     start=False, stop=False)
                    nc.tensor.matmul(px, zs1[:, 0:128], f1_t[:, 1, :],
                                     start=False, stop=False)
                    nc.tensor.matmul(px, zs1[:, 128:256], f2_t[:, 1, :],
                                     start=False, stop=True)
                    xsb = xtp.tile([128, 258], F32)
                    nc.vector.tensor_copy(xsb, px)
                    nc.sync.dma_start(
                        str_xr.rearrange("c (u v) -> c u v", v=Wf)[c, :, :],
                        xsb[:, 0:Wf])
                    nc.sync.dma_start(
                        str_xi.rearrange("c (u v) -> c u v", v=Wf)[c, :, :],
                        xsb[:, Wf:258])

                # ---- einsum over point chunks
                for ci, (s, e) in enumerate(CH):
                    n = e - s
                    exr = exp_.tile([96, 512], F32R, name="exr")
                    exi = exp_.tile([96, 512], F32R, name="exi")
                    nc.gpsimd.dma_start(exr[:, 0:n], str_xr[:, s:e])
                    nc.gpsimd.dma_start(exi[:, 0:n], str_xi[:, s:e])
                    if ci == 0:
                        nc.vector.tensor_add(exr[:, 0:1], exr[:, 0:1],
                                             bdc_t[:, 0:1])
                    pr1 = pse.tile([96, 512], F32, name="e")
                    pi1 = pse.tile([96, 512], F32, name="e")
                    nc.tensor.matmul(pr1[:, 0:n], wt["w1r"], exr[:, 0:n],
                                     start=True, stop=False)
                    nc.tensor.matmul(pr1[:, 0:n], wt["w1in"], exi[:, 0:n],
                                     start=False, stop=True)
                    nc.tensor.matmul(pi1[:, 0:n], wt["w1i"], exr[:, 0:n],
                                     start=True, stop=False)
                    nc.tensor.matmul(pi1[:, 0:n], wt["w1r"], exi[:, 0:n],
                                     start=False, stop=True)
                    r1 = r12p.tile([96, 512], F32R, name="r1")
                    i1 = r12p.tile([96, 512], F32R, name="i1")
                    nc.scalar.activation(r1[:, 0:n], pr1[:, 0:n], AF.Relu,
                                         bias=bt["b1r"][:, 0:1], scale=1.0)
                    nc.scalar.activation(i1[:, 0:n], pi1[:, 0:n], AF.Relu,
                                         bias=bt["b1i"][:, 0:1], scale=1.0)
                    pr2 = pse.tile([96, 512], F32, name="e")
                    pi2 = pse.tile([96, 512], F32, name="e")
                    nc.tensor.matmul(pr2[:, 0:n], wt["w2r"], r1[:, 0:n],
                                     start=True, stop=False)
                    nc.tensor.matmul(pr2[:, 0:n], wt["w2in"], i1[:, 0:n],
                                     start=False, stop=True)
                    nc.tensor.matmul(pi2[:, 0:n], wt["w2i"], r1[:, 0:n],
                                     start=True, stop=False)
                    nc.tensor.matmul(pi2[:, 0:n], wt["w2r"], i1[:, 0:n],
                                     start=False, stop=True)
                    r2 = r12p.tile([96, 512], F32, name="r2")
                    i2 = r12p.tile([96, 512], F32, name="i2")
                    nc.scalar.activation(r2[:, 0:n], pr2[:, 0:n], AF.Relu,
                                         bias=bt["b2r"][:, 0:1], scale=1.0)
                    nc.scalar.activation(i2[:, 0:n], pi2[:, 0:n], AF.Relu,
                                         bias=bt["b2i"][:, 0:1], scale=1.0)
                    nc.sync.dma_start(str_r2[:, s:e], r2[:, 0:n])
                    nc.sync.dma_start(str_i2[:, s:e], i2[:, 0:n])

                # ---- inverse DFT per channel
                for c in range(BS):
                    xr = invp.tile([128, Wf], F32R, name="ixr")
                    xi = invp.tile([128, Wf], F32R, name="ixi")
                    nc.gpsimd.dma_start(
                        xr, str_r2.rearrange("c (u v) -> c u v", v=Wf)[c, :, :])
                    nc.gpsimd.dma_start(
                        xi, str_i2.rearrange("c (u v) -> c u v", v=Wf)[c, :, :])
                    pab = pse.tile([128, 512], F32, name="e")
                    nc.tensor.matmul(pab[:, 0:256], xr[:, 0:128], cs_t,
                                     start=True, stop=True)
                    nc.tensor.matmul(pab[:, 256:512], xi[:, 0:128], cs_t,
                                     start=True, stop=True)
                    pn1 = pse.tile([1, 256], F32, name="e")
                    pn2 = pse.tile([1, 256], F32, name="e")
                    nc.tensor.matmul(pn1, xr[:, 128:129], cs_t,
                                     start=True, stop=True)
                    nc.tensor.matmul(pn2, xi[:, 128:129], cs_t,
                                     start=True, stop=True)
                    absb = invp.tile([128, 512], F32, name="absb")
                    nc.vector.tensor_copy(absb, pab)
                    nsb = invp.tile([1, 512], F32, name="nsb")
                    nc.scalar.copy(nsb[:, 0:256], pn1)
                    nc.scalar.copy(nsb[:, 256:512], pn2)
                    ar = invp.tile([128, 128], F32R, name="ar")
                    ai = invp.tile([128, 128], F32R, name="ai")
                    arn = invp.tile([1, 128], F32R, name="arn")
                    nc.vector.tensor_sub(ar, absb[:, 0:128], absb[:, 384:512])
                    nc.vector.tensor_add(ai, absb[:, 256:384], absb[:, 128:256])
                    nc.vector.tensor_sub(arn, nsb[0:1, 0:128], nsb[0:1, 384:512])
                    py = pse.tile([128, 256], F32, name="e")
                    nc.tensor.matmul(py, ar, cw_t, start=True, stop=False)
                    nc.tensor.matmul(py, ai, sw_t, start=False, stop=False)
                    nc.tensor.matmul(py, arn, cwn_t, start=False, stop=True)
                    yt = ytp.tile([128, 256], F32)
                    nc.vector.tensor_copy(yt, py)
                    nc.sync.dma_start(y[un, c, :, :], yt)
    nc.compile()
    return nc


# ---------------------------------------------------------------- phase 3
def build_phase3():
    nc = bacc.Bacc(None, target_bir_lowering=False)
    h2s = nc.dram_tensor("h2s", [NB, BS, HSLAB, W], F32, kind="ExternalInput")
    xs = nc.dram_tensor("xs", [TPC, D], F32, kind="ExternalInput")
    fc1w = nc.dram_tensor("fc1w", [D, HID], F32, kind="ExternalInput")
    fc2w = nc.dram_tensor("fc2w", [HID, D], F32, kind="ExternalInput")
    fc1b = nc.dram_tensor("fc1b", [HID, 1], F32, kind="ExternalInput")
    fc2b = nc.dram_tensor("fc2b", [1, D], F32, kind="ExternalInput")
    n2g = nc.dram_tensor("n2g", [NB, BS, 1], F32, kind="ExternalInput")
    n2b = nc.dram_tensor("n2b", [NB, BS, 1], F32, kind="ExternalInput")
    out = nc.dram_tensor("out", [TPC, D], F32, kind="ExternalOutput")
    import os
    DBG = bool(int(os.environ.get("K3DBG", "0")))
    if DBG:
        dbg_mu = nc.dram_tensor("dbg_mu", [1, TG], F32, kind="ExternalOutput")
        dbg_rstd = nc.dram_tensor("dbg_rstd", [1, TG], F32, kind="ExternalOutput")
        dbg_nt = nc.dram_tensor("dbg_nt", [96, NB, TG], F32, kind="ExternalOutput")
        dbg_g1 = nc.dram_tensor("dbg_g1", [128, 24, TG], F32, kind="ExternalOutput")

    NG = TPC // TG  # 16 groups
    with tile.TileContext(nc) as tc:
        with tc.tile_pool(name="single", bufs=1) as single, \
             tc.tile_pool(name="w1s", bufs=1) as w1s, \
             tc.tile_pool(name="w2s", bufs=4) as w2s, \
             tc.tile_pool(name="h2r", bufs=1) as h2rp, \
             tc.tile_pool(name="sq", bufs=2) as sqp, \
             tc.tile_pool(name="nt", bufs=1) as ntp, \
             tc.tile_pool(name="g1", bufs=1) as g1p, \
             tc.tile_pool(name="xo", bufs=1) as xop, \
             tc.tile_pool(name="stat", bufs=1) as statp, \
             tc.tile_pool(name="tmp", bufs=2) as tmpp, \
             tc.tile_pool(name="ps_a", bufs=3, space="PSUM") as ps_a, \
             tc.tile_pool(name="ps_o", bufs=1, space="PSUM") as ps_o:
            ones96f = single.tile([96, 1], F32)
            nc.vector.memset(ones96f, 1.0)
            ones96 = single.tile([96, 1], F32R)
            nc.vector.tensor_copy(ones96, ones96f)
            ones1f = single.tile([1, 96], F32)
            nc.vector.memset(ones1f, 1.0)
            ones1 = single.tile([1, 96], F32R)
            nc.vector.tensor_copy(ones1, ones1f)
            epst = single.tile([1, 1], F32)
            nc.vector.memset(epst, EPS)
            fc2bB = single.tile([128, D], F32)
            nc.gpsimd.dma_start(fc2bB, fc2b[:, :].broadcast_to((128, D)))
            fc1b_t = single.tile([128, 24, 1], F32)
            nc.sync.dma_start(
                fc1b_t, fc1b[:, :].rearrange("(k p) o -> p k o", p=128))
            n2g_t = single.tile([96, 8, 1], F32)
            nc.sync.dma_start(n2g_t,
                              n2g[:, :, :].rearrange("b c o -> c b o"))
            n2b_t = single.tile([96, 8, 1], F32)
            nc.sync.dma_start(n2b_t,
                              n2b[:, :, :].rearrange("b c o -> c b o"))

            for g in range(NG):
                h2r = h2rp.tile([96, NB, TG], F32R, name="h2r")
                nc.gpsimd.dma_start(
                    h2r, h2s[:, :, 2 * g:2 * g + 2, :]
                    .rearrange("b c h w -> c b (h w)"))
                # stats via ones-matmuls
                pmu = ps_a.tile([1, TG], F32, name="ph")
                pmu2 = ps_a.tile([1, TG], F32, name="ph")
                for blk in range(NB):
                    nc.tensor.matmul(pmu, ones96, h2r[:, blk, :],
                                     start=(blk == 0), stop=(blk == NB - 1))
                for blk in range(NB):
                    sq = sqp.tile([96, TG], F32R, name="sq")
                    nc.scalar.activation(sq, h2r[:, blk, :], AF.Square,
                                         scale=1.0)
                    nc.tensor.matmul(pmu2, ones96, sq,
                                     start=(blk == 0), stop=(blk == NB - 1))
                mu = statp.tile([1, TG], F32, name="mu")
                nc.vector.tensor_scalar_mul(mu, pmu, 1.0 / D)
                va = statp.tile([1, TG], F32, name="va")
                vb = statp.tile([1, TG], F32, name="vb")
                nc.vector.tensor_scalar_mul(va, pmu2, 1.0 / D)
                nc.vector.tensor_mul(vb, mu, mu)
                nc.vector.tensor_sub(va, va, vb)
                nc.scalar.activation(va, va, AF.Sqrt,
                                     bias=epst[0:1, 0:1], scale=1.0)
                nc.vector.reciprocal(va, va)
                mu_r = statp.tile([1, TG], F32R, name="mu_r")
                nc.vector.tensor_copy(mu_r, mu)
                rstd_r = statp.tile([1, TG], F32R, name="rstd_r")
                nc.vector.tensor_copy(rstd_r, va)
                pmub = ps_a.tile([96, TG], F32, name="ph")
                nc.tensor.matmul(pmub, ones1, mu_r, start=True, stop=True)
                prstdb = ps_a.tile([96, TG], F32, name="ph")
                nc.tensor.matmul(prstdb, ones1, rstd_r, start=True, stop=True)
                mub = statp.tile([96, TG], F32R, name="mub")
                nc.vector.tensor_copy(mub, pmub)
                rstdb = statp.tile([96, TG], F32R, name="rstdb")
                nc.vector.tensor_copy(rstdb, prstdb)

                nt = ntp.tile([96, NB, TG], F32R, name="nt")
                for blk in range(NB):
                    nc.vector.tensor_sub(nt[:, blk, :], h2r[:, blk, :], mub)
                    nc.vector.tensor_mul(nt[:, blk, :], nt[:, blk, :], rstdb)
                    nc.scalar.activation(nt[:, blk, :], nt[:, blk, :],
                                         AF.Identity,
                                         bias=n2b_t[:, blk, 0:1],
                                         scale=n2g_t[:, blk, 0:1])
                if DBG and g == 0:
                    nc.sync.dma_start(dbg_mu[:, :], mu)
                    nc.sync.dma_start(dbg_rstd[:, :], va)
                    nc.gpsimd.dma_start(dbg_nt[:, :, :], nt)
                # fc1 + gelu -> g1T  (weights streamed in halves)
                g1 = g1p.tile([128, 24, TG], F32R, name="g1")
                for half in range(2):
                    f1t = w1s.tile([96, NB, HID // 2], F32R, name="f1t")
                    nc.gpsimd.dma_start(
                        f1t, fc1w[:, half * (HID // 2):(half + 1) * (HID // 2)]
                        .rearrange("(b c) h -> c b h", c=BS))
                    for hh in range(12):
                        hc = half * 12 + hh
                        ph = ps_a.tile([128, TG], F32, name="ph")
                        for blk in range(NB):
                            nc.tensor.matmul(
                                ph, f1t[:, blk, hh * 128:(hh + 1) * 128],
                                nt[:, blk, :], start=(blk == 0),
                                stop=(blk == NB - 1))
                        nc.scalar.activation(g1[:, hc, :], ph, AF.Gelu,
                                             bias=fc1b_t[:, hc, 0:1],
                                             scale=1.0)
                if DBG and g == 0:
                    nc.gpsimd.dma_start(dbg_g1[:, :, :], g1[:, :, :])
                # fc2 + bias + residual
                xt = xop.tile([128, 4, D], F32, name="xt")
                nc.sync.dma_start(
                    xt, xs[g * TG:(g + 1) * TG, :]
                    .rearrange("(m p) d -> p m d", p=128))
                ot = xop.tile([128, 4, D], F32, name="ot")
                for npass, (d0, d1) in enumerate([(0, 512), (512, 768)]):
                    nw = d1 - d0
                    po = ps_o.tile([128, 4, 512], F32, name="po")
                    for k in range(24):
                        f2t = w2s.tile([128, 512], F32R, name="f2t")
                        nc.gpsimd.dma_start(f2t[:, 0:nw],
                                            fc2w[k * 128:(k + 1) * 128, d0:d1])
                        for m in range(4):
                            nc.tensor.matmul(
                                po[:, m, 0:nw],
                                g1[:, k, m * 128:(m + 1) * 128],
                                f2t[:, 0:nw],
                                start=(k == 0), stop=(k == 23))
                    for m in range(4):
                        tmp = tmpp.tile([128, 512], F32, name="tmp0")
                        nc.vector.tensor_add(tmp[:, 0:nw], po[:, m, 0:nw],
                                             fc2bB[:, d0:d1])
                        nc.vector.tensor_add(ot[:, m, d0:d1], tmp[:, 0:nw],
                                             xt[:, m, d0:d1])
                nc.sync.dma_start(
                    out[g * TG:(g + 1) * TG, :]
                    .rearrange("(m p) d -> p m d", p=128), ot)
    nc.compile()
    return nc


# ---------------------------------------------------------------- host glue
def _get(name, builder):
    if name not in _programs:
        _programs[name] = builder()
    return _programs[name]


def run_phase1(x):
    nc = _get("p1", build_phase1)
    xf = np.ascontiguousarray(x.reshape(2 * HW, D), dtype=np.float32)
    in_maps = [{"xs": xf[c * TPC:(c + 1) * TPC]} for c in range(NCORES)]
    res = run_bass_kernel_spmd(nc, in_maps, core_ids=list(range(NCORES)))
    slabs = [r["o"] for r in res.results]  # each [NB, BS, 32, W]
    h1cm = np.empty((2, NB, BS, H, W), np.float32)
    for c in range(NCORES):
        b, hi = divmod(c, 4)
        h1cm[b, :, :, hi * HSLAB:(hi + 1) * HSLAB, :] = slabs[c]
    return h1cm


def _phase2_inmaps(h1cm, M, inp):
    g = inp["norm1_g"].astype(np.float32)
    b = inp["norm1_b"].astype(np.float32)
    w1, w2 = inp["w1"].astype(np.float32), inp["w2"].astype(np.float32)
    b1, b2 = inp["b1"].astype(np.float32), inp["b2"].astype(np.float32)
    units = [(u // NB, u % NB) for u in range(16)]
    in_maps = []
    for c in range(NCORES):
        us = units[2 * c:2 * c + 2]
        blks = [blk for _, blk in us]
        m = {"data": np.stack([h1cm[bb, blk] for bb, blk in us]),
             "ecat": M["Ecat"], "f1": M["F1"], "f2": M["F2"], "cs": M["CS"],
             "cw": M["cw"], "sw": M["sw"], "cwn": M["cwn"]}
        gs = np.stack([g[blk * BS:(blk + 1) * BS] for blk in blks])
        m["w1r"] = np.ascontiguousarray(gs[:, :, None] * w1[0][blks])
        m["w1i"] = np.ascontiguousarray(gs[:, :, None] * w1[1][blks])
        m["w1in"] = np.ascontiguousarray(-m["w1i"])
        m["w2r"] = np.ascontiguousarray(w2[0][blks])
        m["w2i"] = np.ascontiguousarray(w2[1][blks])
        m["w2in"] = np.ascontiguousarray(-m["w2i"])
        m["b1r"] = np.ascontiguousarray(b1[0][blks][:, :, None])
        m["b1i"] = np.ascontiguousarray(b1[1][blks][:, :, None])
        m["b2r"] = np.ascontiguousarray((b2[0] - LAM)[blks][:, :, None])
        m["b2i"] = np.ascontiguousarray((b2[1] - LAM)[blks][:, :, None])
        m["bdc"] = np.ascontiguousarray(
            np.stack([b[blk * BS:(blk + 1) * BS] for blk in blks])[:, :, None]
            * SQHW)
        in_maps.append(m)
    return in_maps


def run_phase2(h1cm, M, inp):
    nc = _get("p2", build_phase2)
    in_maps = _phase2_inmaps(h1cm, M, inp)
    res = run_bass_kernel_spmd(nc, in_maps, core_ids=list(range(NCORES)))
    units = [(u // NB, u % NB) for u in range(16)]
    h2cm = np.empty((2, NB, BS, H, W), np.float32)
    for c in range(NCORES):
        for j, (bb, blk) in enumerate(units[2 * c:2 * c + 2]):
            h2cm[bb, blk] = res.results[c]["y"][j]
    return h2cm


def _phase3_inmaps(h2cm, xf, inp):
    fc1w = np.ascontiguousarray(inp["fc1_w"], np.float32)
    fc2w = np.ascontiguousarray(inp["fc2_w"], np.float32)
    fc1b = np.ascontiguousarray(inp["fc1_b"], np.float32)[:, None]
    fc2b = np.ascontiguousarray(inp["fc2_b"], np.float32)[None, :]
    n2g = np.ascontiguousarray(inp["norm2_g"], np.float32).reshape(NB, BS, 1)
    n2b = np.ascontiguousarray(inp["norm2_b"], np.float32).reshape(NB, BS, 1)
    in_maps = []
    for c in range(NCORES):
        b, hi = divmod(c, 4)
        in_maps.append({
            "h2s": np.ascontiguousarray(
                h2cm[b, :, :, hi * HSLAB:(hi + 1) * HSLAB, :]),
            "xs": xf[c * TPC:(c + 1) * TPC],
            "fc1w": fc1w, "fc2w": fc2w, "fc1b": fc1b, "fc2b": fc2b,
            "n2g": n2g, "n2b": n2b})
    return in_maps


def run_phase3(h2cm, x, inp):
    nc = _get("p3", build_phase3)
    xf = np.ascontiguousarray(x.reshape(2 * HW, D), dtype=np.float32)
    in_maps = _phase3_inmaps(h2cm, xf, inp)
    res = run_bass_kernel_spmd(nc, in_maps, core_ids=list(range(NCORES)))
    out = np.concatenate([r["out"] for r in res.results], axis=0)
    return out.reshape(2, HW, D)


def kernel(**inputs):
    inp = {k: np.asarray(v) for k, v in inputs.items()}
    x = inp["x"].astype(np.float32)
    M = build_mats()
    h1cm = run_phase1(x)
    h2cm = run_phase2(h1cm, M, inp)
    return run_phase3(h2cm, x, inp)


if __name__ == "__main__":
    rng = np.random.default_rng(0)
    demo = {"x": rng.standard_normal((2, HW, D), dtype=np.float32)}
    print("kernel module ok")



# revision 2
# speedup vs baseline: 1.0375x; 1.0375x over previous
"""AFNO block (nn_Block_32109175505281) on 8 Trainium2 NeuronCores.

One fused NEFF per batch-image (the two batch elements are independent:
LN/MLP are per-token, the FFT mixes only within a batch). kernel() makes
two async launches of the SAME compiled program, so batch 1's host-side
quantization + H2D overlap batch 0's execution + D2H.

The axon tunnel is slow (~80 MB/s H2D, ~40 MB/s D2H), so wire bytes
dominate wall time:
  - x travels int8 (50MB total), quantized per token row by absmax on
    host; no scales are shipped because LayerNorm is invariant to
    per-token scaling and the residual uses exact f32 x on host,
  - the device returns only the MLP delta as int8 with per-partition-row
    scales (f32->int8 cast is round-to-nearest, verified); the f32
    residual add (out = x + delta) happens on host,
  - all static weights (fc1/fc2/norm2/DFT matrices) are baked into the
    NEFF as inline Const tensors (zero wire cost); the compiled-program
    cache is keyed by their content hash,
  - per-core spectral-MLP weights travel once per weight-set as a small
    sharded device-resident array.

Per-launch sharding (one batch image [128, 256, 768]):
  Phase A token-sharded: core c owns h-rows [16c, 16c+16) (a contiguous
     slab of x -> no host reshuffle). LN1 (no affine; g folded into
     einsum weights, b via DC fix) + PE transpose -> channel-major slabs
     scattered into the A2A buffer [dst8, h16, ch96, w256].
  A2A #1 (8-core): core c ends up with channel block c, full H.
  Phase B: matmul-DFT rfft2, 2-layer block-diagonal complex MLP (relu,
     softshrink folded into relu bias), matmul-DFT irfft2.
  A2A #2: back to token sharding ([src8=blk, h16, ch96, w256]).
  Phase C: LN2 (stats via ones-matmul in transposed layout), MLP
     768->3072->3072->768 (exact GELU) -> int8 delta + scales out.
"""
import sys
import functools
import hashlib
import numpy as np

sys.path.insert(0, '/opt/trn_rl_repo')

import jax
from jax.sharding import Mesh, PartitionSpec as P, NamedSharding

import concourse.bacc as bacc
import concourse.tile as tile
import concourse.mybir as mybir
from concourse.bass2jax import bass_jit, bass_shard_map
from concourse.masks import make_identity

F32 = mybir.dt.float32
F32R = mybir.dt.float32r
I8 = mybir.dt.int8
AF = mybir.ActivationFunctionType

H, W, NB, BS, D = 128, 256, 8, 96, 768
Wf = W // 2 + 1        # 129
HW = H * W             # 32768
HID = 4 * D            # 3072
LAM = 0.01
EPS = 1e-5
SQHW = float(np.sqrt(H * W))
NCORES = 8
TPC = HW // NCORES      # tokens per core per launch = 4096
HSLAB = H // NCORES     # 16 h-rows per core per launch
Pts = H * Wf            # 16512 frequency points
TG = 512                # phase-C token group
NG = TPC // TG          # 8 phase-C token groups
RG8 = [[0, 1, 2, 3, 4, 5, 6, 7]]

# wp column layout: 6 [96,96] mats then 5 [96,1] bias cols
WCOL = {n: 96 * i for i, n in
        enumerate(["w1r", "w1i", "w1in", "w2r", "w2i", "w2in"])}
BCOL = {n: 576 + i for i, n in
        enumerate(["b1r", "b1i", "b2r", "b2i", "bdc"])}
WPW = 581

_programs = {}
_wp_cache = {}
_mesh = None


def _get_mesh():
    global _mesh
    if _mesh is None:
        _mesh = Mesh(np.asarray(jax.devices()[:NCORES]), ("core",))
    return _mesh


# ---------------------------------------------------------------- matrices
def build_mats():
    f64 = np.float64
    h = np.arange(H, dtype=f64)
    u = np.arange(H, dtype=f64)
    w = np.arange(W, dtype=f64)
    v = np.arange(Wf, dtype=f64)
    th = 2 * np.pi * np.outer(h, u) / H
    Ecat = np.concatenate([np.cos(th), -np.sin(th)], axis=1) / SQHW  # [128,256]
    tw = 2 * np.pi * np.outer(w, v) / W
    Fr, Fs = np.cos(tw), np.sin(tw)
    Fcat1 = np.concatenate([Fr, -Fs], axis=1)  # [256,258]
    Fcat2 = np.concatenate([Fs, Fr], axis=1)
    thi = 2 * np.pi * np.outer(u, h) / H
    CS = np.concatenate([np.cos(thi), np.sin(thi)], axis=1) / SQHW   # [128,256]
    mu = np.ones(Wf); mu[1:W // 2] = 2.0
    twi = 2 * np.pi * np.outer(v, w) / W
    cw_full = mu[:, None] * np.cos(twi)
    sw_full = -mu[:, None] * np.sin(twi)
    c = lambda a: np.ascontiguousarray(a, dtype=np.float32)
    return dict(Ecat=c(Ecat),
                F1=c(Fcat1.reshape(2, 128, 258).transpose(1, 0, 2)),  # [128,2,258]
                F2=c(Fcat2.reshape(2, 128, 258).transpose(1, 0, 2)),
                CS=c(CS), cw=c(cw_full[:128]), sw=c(sw_full[:128]),
                cwn=c(cw_full[128:129]))


# ---------------------------------------------------------------- builder
def make_fused(consts, bi):
    """consts: numpy arrays baked into the NEFF; bi: which batch half of the
    device-resident [2, 4096, 768] input slab this program reads."""

    @functools.partial(bass_jit, num_devices=NCORES)
    def fused(nc, xs2, wp):
        # xs2: [2, 4096, 768] int8 (16 h-rows of both batches), wp: [96, 581]
        xs = xs2[bi]
        out = nc.dram_tensor("out", [TPC, D], I8, kind="ExternalOutput")
        out_s = nc.dram_tensor("out_s", [NG * 128, 1], F32,
                               kind="ExternalOutput")
        # DFT mats as consts
        ecat = nc.inline_tensor(consts["Ecat"], name="ecat")
        f1 = nc.inline_tensor(consts["F1"], name="f1")
        f2 = nc.inline_tensor(consts["F2"], name="f2")
        cs = nc.inline_tensor(consts["CS"], name="cs")
        cw = nc.inline_tensor(consts["cw"], name="cw")
        sw = nc.inline_tensor(consts["sw"], name="sw")
        cwn = nc.inline_tensor(consts["cwn"], name="cwn")
        # MLP weights as consts (pre-rearranged)
        fc1wt = nc.inline_tensor(consts["fc1wt"], name="fc1wt")  # [96,8,3072]
        fc2wc = nc.inline_tensor(consts["fc2w"], name="fc2wc")   # [3072,768]
        fc1bt = nc.inline_tensor(consts["fc1bt"], name="fc1bt")  # [128,24,1]
        fc2bc = nc.inline_tensor(consts["fc2b"], name="fc2bc")   # [1,768]
        n2gt = nc.inline_tensor(consts["n2gt"], name="n2gt")     # [96,8,1]
        n2bt = nc.inline_tensor(consts["n2bt"], name="n2bt")     # [96,8,1]

        with tile.TileContext(nc) as tc:
            with tc.tile_pool(name="dram", bufs=1, space="DRAM") as dramp:
                # A2A buffers: [core8, h16, ch96, w256] so (core h) merges
                A = dramp.tile([NCORES, HSLAB, BS, W], F32, name="A")
                Bb = dramp.tile([NCORES, HSLAB, BS, W], F32, name="Bb")
                C = dramp.tile([NCORES, HSLAB, BS, W], F32, name="C")
                Dd = dramp.tile([NCORES, HSLAB, BS, W], F32, name="Dd")

                # ======================================== phase A: LN1 + scatter
                with tc.tile_pool(name="asingle", bufs=1) as single, \
                     tc.tile_pool(name="xb", bufs=3) as xbp, \
                     tc.tile_pool(name="xt", bufs=3) as xtp, \
                     tc.tile_pool(name="st", bufs=3) as stp, \
                     tc.tile_pool(name="ot", bufs=6) as otp, \
                     tc.tile_pool(name="ps", bufs=6, space="PSUM") as psp:
                    ident = single.tile([128, 128], F32)
                    make_identity(nc, ident)
                    epst = single.tile([128, 1], F32)
                    nc.vector.memset(epst, EPS)

                    for t in range(TPC // 128):  # 32 tiles
                        hl, wc = t // 2, t % 2
                        xb = xbp.tile([128, D], I8)
                        nc.sync.dma_start(xb, xs[t * 128:(t + 1) * 128, :])
                        xt = xtp.tile([128, D], F32)
                        nc.vector.tensor_copy(xt, xb)
                        st = stp.tile([128, 3, 6], F32)
                        for sg in range(3):
                            nc.vector.bn_stats(st[:, sg, :],
                                               xt[:, sg * 256:(sg + 1) * 256])
                        mv = stp.tile([128, 2], F32)
                        nc.vector.bn_aggr(mv, st)
                        rstd = stp.tile([128, 1], F32)
                        nc.scalar.activation(rstd, mv[:, 1:2], AF.Sqrt,
                                             bias=epst[:, 0:1], scale=1.0)
                        nc.vector.reciprocal(rstd, rstd)
                        nc.vector.tensor_scalar(out=xt, in0=xt,
                                                scalar1=mv[:, 0:1], scalar2=rstd,
                                                op0=mybir.AluOpType.subtract,
                                                op1=mybir.AluOpType.mult)
                        for blk in range(NB):
                            pt = psp.tile([96, 128], F32, name="pt")
                            nc.tensor.transpose(
                                pt, xt[:, blk * BS:(blk + 1) * BS], ident)
                            ot = otp.tile([96, 128], F32)
                            if blk % 2 == 0:
                                nc.vector.tensor_copy(ot, pt)
                            else:
                                nc.scalar.copy(ot, pt)
                            nc.sync.dma_start(
                                A[blk, hl, :, wc * 128:(wc + 1) * 128], ot)

                # ======================================== A2A #1
                nc.gpsimd.collective_compute(
                    "AllToAll", mybir.AluOpType.bypass, replica_groups=RG8,
                    ins=[A.opt()], outs=[Bb.opt()])
                # Bb[i, h16, ch, w]: rows [16i,16i+16) of my channel block

                # ======================================== phase B: spectral MLP
                CH = [(s, min(s + 512, Pts)) for s in range(0, Pts, 512)]
                with tc.tile_pool(name="bsingle", bufs=1) as single, \
                     tc.tile_pool(name="uw", bufs=1) as uwp, \
                     tc.tile_pool(name="din", bufs=3) as dinp, \
                     tc.tile_pool(name="zt", bufs=4) as ztp, \
                     tc.tile_pool(name="xt2", bufs=3) as xtp, \
                     tc.tile_pool(name="ex", bufs=4) as exp_, \
                     tc.tile_pool(name="r12", bufs=4) as r12p, \
                     tc.tile_pool(name="inv", bufs=4) as invp, \
                     tc.tile_pool(name="yt", bufs=4) as ytp, \
                     tc.tile_pool(name="psa", bufs=4, space="PSUM") as psa, \
                     tc.tile_pool(name="pse", bufs=4, space="PSUM") as pse, \
                     tc.tile_pool(name="bdram", bufs=1, space="DRAM") as bdram:
                    ecat_t = single.tile([128, 256], F32R)
                    nc.gpsimd.dma_start(ecat_t, ecat[:, :])
                    f1_t = single.tile([128, 2, 258], F32R)
                    nc.gpsimd.dma_start(f1_t, f1[:, :, :])
                    f2_t = single.tile([128, 2, 258], F32R)
                    nc.gpsimd.dma_start(f2_t, f2[:, :, :])
                    cs_t = single.tile([128, 256], F32R)
                    nc.gpsimd.dma_start(cs_t, cs[:, :])
                    cw_t = single.tile([128, 256], F32R)
                    nc.gpsimd.dma_start(cw_t, cw[:, :])
                    sw_t = single.tile([128, 256], F32R)
                    nc.gpsimd.dma_start(sw_t, sw[:, :])
                    cwn_t = single.tile([1, 256], F32R)
                    nc.gpsimd.dma_start(cwn_t, cwn[:, :])
                    # my block's weights
                    wt = {}
                    for name, off in WCOL.items():
                        wt[name] = uwp.tile([96, 96], F32R, name=name)
                        nc.gpsimd.dma_start(wt[name], wp[:, off:off + 96])
                    bt = {}
                    for name in ["b1r", "b1i", "b2r", "b2i"]:
                        off = BCOL[name]
                        bt[name] = uwp.tile([96, 1], F32, name=name)
                        nc.sync.dma_start(bt[name], wp[:, off:off + 1])
                    bdc_t = uwp.tile([96, 1], F32R, name="bdc")
                    nc.gpsimd.dma_start(bdc_t, wp[:, BCOL["bdc"]:BCOL["bdc"] + 1])

                    str_xr = bdram.tile([BS, Pts], F32, name="sxr")
                    str_xi = bdram.tile([BS, Pts], F32, name="sxi")
                    str_r2 = bdram.tile([BS, Pts], F32, name="sr2")
                    str_i2 = bdram.tile([BS, Pts], F32, name="si2")

                    # ---- forward DFT per channel
                    for c in range(BS):
                        din = dinp.tile([128, 256], F32R)
                        nc.gpsimd.dma_start(
                            din, Bb[:, :, c, :].rearrange("i h w -> (i h) w"))
                        z0 = psa.tile([128, 256], F32, name="a")
                        z1 = psa.tile([128, 256], F32, name="a")
                        nc.tensor.matmul(z0, din[:, 0:128], ecat_t,
                                         start=True, stop=True)
                        nc.tensor.matmul(z1, din[:, 128:256], ecat_t,
                                         start=True, stop=True)
                        zs0 = ztp.tile([128, 256], F32R, name="zs")
                        zs1 = ztp.tile([128, 256], F32R, name="zs")
                        nc.vector.tensor_copy(zs0, z0)
                        nc.scalar.copy(zs1, z1)
                        px = psa.tile([128, 258], F32, name="a")
                        nc.tensor.matmul(px, zs0[:, 0:128], f1_t[:, 0, :],
                                         start=True, stop=False)
                        nc.tensor.matmul(px, zs0[:, 128:256], f2_t[:, 0, :],
                                         start=False, stop=False)
                        nc.tensor.matmul(px, zs1[:, 0:128], f1_t[:, 1, :],
                                         start=False, stop=False)
                        nc.tensor.matmul(px, zs1[:, 128:256], f2_t[:, 1, :],
                                         start=False, stop=True)
                        xsb = xtp.tile([128, 258], F32)
                        nc.vector.tensor_copy(xsb, px)
                        nc.sync.dma_start(
                            str_xr.rearrange("c (u v) -> c u v", v=Wf)[c, :, :],
                            xsb[:, 0:Wf])
                        nc.sync.dma_start(
                            str_xi.rearrange("c (u v) -> c u v", v=Wf)[c, :, :],
                            xsb[:, Wf:258])

                    # ---- einsum over point chunks
                    for ci, (s, e) in enumerate(CH):
                        n = e - s
                        exr = exp_.tile([96, 512], F32R, name="exr")
                        exi = exp_.tile([96, 512], F32R, name="exi")
                        nc.gpsimd.dma_start(exr[:, 0:n], str_xr[:, s:e])
                        nc.gpsimd.dma_start(exi[:, 0:n], str_xi[:, s:e])
                        if ci == 0:
                            nc.vector.tensor_add(exr[:, 0:1], exr[:, 0:1],
                                                 bdc_t[:, 0:1])
                        pr1 = pse.tile([96, 512], F32, name="e")
                        pi1 = pse.tile([96, 512], F32, name="e")
                        nc.tensor.matmul(pr1[:, 0:n], wt["w1r"], exr[:, 0:n],
                                         start=True, stop=False)
                        nc.tensor.matmul(pr1[:, 0:n], wt["w1in"], exi[:, 0:n],
                                         start=False, stop=True)
                        nc.tensor.matmul(pi1[:, 0:n], wt["w1i"], exr[:, 0:n],
                                         start=True, stop=False)
                        nc.tensor.matmul(pi1[:, 0:n], wt["w1r"], exi[:, 0:n],
                                         start=False, stop=True)
                        r1 = r12p.tile([96, 512], F32R, name="r1")
                        i1 = r12p.tile([96, 512], F32R, name="i1")
                        nc.scalar.activation(r1[:, 0:n], pr1[:, 0:n], AF.Relu,
                                             bias=bt["b1r"][:, 0:1], scale=1.0)
                        nc.scalar.activation(i1[:, 0:n], pi1[:, 0:n], AF.Relu,
                                             bias=bt["b1i"][:, 0:1], scale=1.0)
                        pr2 = pse.tile([96, 512], F32, name="e")
                        pi2 = pse.tile([96, 512], F32, name="e")
                        nc.tensor.matmul(pr2[:, 0:n], wt["w2r"], r1[:, 0:n],
                                         start=True, stop=False)
                        nc.tensor.matmul(pr2[:, 0:n], wt["w2in"], i1[:, 0:n],
                                         start=False, stop=True)
                        nc.tensor.matmul(pi2[:, 0:n], wt["w2i"], r1[:, 0:n],
                                         start=True, stop=False)
                        nc.tensor.matmul(pi2[:, 0:n], wt["w2r"], i1[:, 0:n],
                                         start=False, stop=True)
                        r2 = r12p.tile([96, 512], F32, name="r2")
                        i2 = r12p.tile([96, 512], F32, name="i2")
                        nc.scalar.activation(r2[:, 0:n], pr2[:, 0:n], AF.Relu,
                                             bias=bt["b2r"][:, 0:1], scale=1.0)
                        nc.scalar.activation(i2[:, 0:n], pi2[:, 0:n], AF.Relu,
                                             bias=bt["b2i"][:, 0:1], scale=1.0)
                        nc.sync.dma_start(str_r2[:, s:e], r2[:, 0:n])
                        nc.sync.dma_start(str_i2[:, s:e], i2[:, 0:n])

                    # ---- inverse DFT per channel
                    for c in range(BS):
                        xr = invp.tile([128, Wf], F32R, name="ixr")
                        xi = invp.tile([128, Wf], F32R, name="ixi")
                        nc.gpsimd.dma_start(
                            xr, str_r2.rearrange("c (u v) -> c u v", v=Wf)[c, :, :])
                        nc.gpsimd.dma_start(
                            xi, str_i2.rearrange("c (u v) -> c u v", v=Wf)[c, :, :])
                        pab = pse.tile([128, 512], F32, name="e")
                        nc.tensor.matmul(pab[:, 0:256], xr[:, 0:128], cs_t,
                                         start=True, stop=True)
                        nc.tensor.matmul(pab[:, 256:512], xi[:, 0:128], cs_t,
                                         start=True, stop=True)
                        pn1 = pse.tile([1, 256], F32, name="e")
                        pn2 = pse.tile([1, 256], F32, name="e")
                        nc.tensor.matmul(pn1, xr[:, 128:129], cs_t,
                                         start=True, stop=True)
                        nc.tensor.matmul(pn2, xi[:, 128:129], cs_t,
                                         start=True, stop=True)
                        absb = invp.tile([128, 512], F32, name="absb")
                        nc.vector.tensor_copy(absb, pab)
                        nsb = invp.tile([1, 512], F32, name="nsb")
                        nc.scalar.copy(nsb[:, 0:256], pn1)
                        nc.scalar.copy(nsb[:, 256:512], pn2)
                        ar = invp.tile([128, 128], F32R, name="ar")
                        ai = invp.tile([128, 128], F32R, name="ai")
                        arn = invp.tile([1, 128], F32R, name="arn")
                        nc.vector.tensor_sub(ar, absb[:, 0:128],
                                             absb[:, 384:512])
                        nc.vector.tensor_add(ai, absb[:, 256:384],
                                             absb[:, 128:256])
                        nc.vector.tensor_sub(arn, nsb[0:1, 0:128],
                                             nsb[0:1, 384:512])
                        py = pse.tile([128, 256], F32, name="e")
                        nc.tensor.matmul(py, ar, cw_t, start=True, stop=False)
                        nc.tensor.matmul(py, ai, sw_t, start=False, stop=False)
                        nc.tensor.matmul(py, arn, cwn_t, start=False, stop=True)
                        yt = ytp.tile([128, 256], F32)
                        nc.vector.tensor_copy(yt, py)
                        nc.sync.dma_start(
                            C[:, :, c, :].rearrange("q h w -> (q h) w"), yt)

                # ======================================== A2A #2
                nc.gpsimd.collective_compute(
                    "AllToAll", mybir.AluOpType.bypass, replica_groups=RG8,
                    ins=[C.opt()], outs=[Dd.opt()])
                # Dd[blk, h16, ch, w] = h2 channel-major for my 16 rows

                # ======================================== phase C: LN2 + MLP
                with tc.tile_pool(name="csingle", bufs=1) as single, \
                     tc.tile_pool(name="w1s", bufs=1) as w1s, \
                     tc.tile_pool(name="w2s", bufs=4) as w2s, \
                     tc.tile_pool(name="h2r", bufs=1) as h2rp, \
                     tc.tile_pool(name="sq", bufs=2) as sqp, \
                     tc.tile_pool(name="nt", bufs=1) as ntp, \
                     tc.tile_pool(name="g1", bufs=1) as g1p, \
                     tc.tile_pool(name="xo", bufs=1) as xop, \
                     tc.tile_pool(name="stat", bufs=1) as statp, \
                     tc.tile_pool(name="tmp", bufs=2) as tmpp, \
                     tc.tile_pool(name="ps_a", bufs=3, space="PSUM") as ps_a, \
                     tc.tile_pool(name="ps_o", bufs=1, space="PSUM") as ps_o:
                    ones96f = single.tile([96, 1], F32)
                    nc.vector.memset(ones96f, 1.0)
                    ones96 = single.tile([96, 1], F32R)
                    nc.vector.tensor_copy(ones96, ones96f)
                    ones1f = single.tile([1, 96], F32)
                    nc.vector.memset(ones1f, 1.0)
                    ones1 = single.tile([1, 96], F32R)
                    nc.vector.tensor_copy(ones1, ones1f)
                    epst = single.tile([1, 1], F32)
                    nc.vector.memset(epst, EPS)
                    fc2bB = single.tile([128, D], F32)
                    nc.gpsimd.dma_start(fc2bB, fc2bc[:, :].broadcast_to((128, D)))
                    fc1b_t = single.tile([128, 24, 1], F32)
                    nc.sync.dma_start(fc1b_t, fc1bt[:, :, :])
                    n2g_t = single.tile([96, 8, 1], F32)
                    nc.sync.dma_start(n2g_t, n2gt[:, :, :])
                    n2b_t = single.tile([96, 8, 1], F32)
                    nc.sync.dma_start(n2b_t, n2bt[:, :, :])

                    for g in range(NG):
                        h2r = h2rp.tile([96, NB, TG], F32R, name="h2r")
                        for r in range(2):
                            nc.gpsimd.dma_start(
                                h2r[:, :, r * 256:(r + 1) * 256],
                                Dd[:, 2 * g + r, :, :]
                                .rearrange("b c w -> c b w"))
                        pmu = ps_a.tile([1, TG], F32, name="ph")
                        pmu2 = ps_a.tile([1, TG], F32, name="ph")
                        for blk in range(NB):
                            nc.tensor.matmul(pmu, ones96, h2r[:, blk, :],
                                             start=(blk == 0), stop=(blk == NB - 1))
                        for blk in range(NB):
                            sq = sqp.tile([96, TG], F32R, name="sq")
                            nc.scalar.activation(sq, h2r[:, blk, :], AF.Square,
                                                 scale=1.0)
                            nc.tensor.matmul(pmu2, ones96, sq,
                                             start=(blk == 0), stop=(blk == NB - 1))
                        mu = statp.tile([1, TG], F32, name="mu")
                        nc.vector.tensor_scalar_mul(mu, pmu, 1.0 / D)
                        va = statp.tile([1, TG], F32, name="va")
                        vb = statp.tile([1, TG], F32, name="vb")
                        nc.vector.tensor_scalar_mul(va, pmu2, 1.0 / D)
                        nc.vector.tensor_mul(vb, mu, mu)
                        nc.vector.tensor_sub(va, va, vb)
                        nc.scalar.activation(va, va, AF.Sqrt,
                                             bias=epst[0:1, 0:1], scale=1.0)
                        nc.vector.reciprocal(va, va)
                        mu_r = statp.tile([1, TG], F32R, name="mu_r")
                        nc.vector.tensor_copy(mu_r, mu)
                        rstd_r = statp.tile([1, TG], F32R, name="rstd_r")
                        nc.vector.tensor_copy(rstd_r, va)
                        pmub = ps_a.tile([96, TG], F32, name="ph")
                        nc.tensor.matmul(pmub, ones1, mu_r, start=True, stop=True)
                        prstdb = ps_a.tile([96, TG], F32, name="ph")
                        nc.tensor.matmul(prstdb, ones1, rstd_r,
                                         start=True, stop=True)
                        mub = statp.tile([96, TG], F32R, name="mub")
                        nc.vector.tensor_copy(mub, pmub)
                        rstdb = statp.tile([96, TG], F32R, name="rstdb")
                        nc.vector.tensor_copy(rstdb, prstdb)

                        nt = ntp.tile([96, NB, TG], F32R, name="nt")
                        for blk in range(NB):
                            nc.vector.tensor_sub(nt[:, blk, :], h2r[:, blk, :], mub)
                            nc.vector.tensor_mul(nt[:, blk, :], nt[:, blk, :],
                                                 rstdb)
                            nc.scalar.activation(nt[:, blk, :], nt[:, blk, :],
                                                 AF.Identity,
                                                 bias=n2b_t[:, blk, 0:1],
                                                 scale=n2g_t[:, blk, 0:1])
                        # fc1 + gelu (weights streamed in halves)
                        g1 = g1p.tile([128, 24, TG], F32R, name="g1")
                        for half in range(2):
                            f1t = w1s.tile([96, NB, HID // 2], F32R, name="f1t")
                            nc.gpsimd.dma_start(
                                f1t, fc1wt[:, :, half * (HID // 2):
                                           (half + 1) * (HID // 2)])
                            for hh in range(12):
                                hc = half * 12 + hh
                                ph = ps_a.tile([128, TG], F32, name="ph")
                                for blk in range(NB):
                                    nc.tensor.matmul(
                                        ph, f1t[:, blk, hh * 128:(hh + 1) * 128],
                                        nt[:, blk, :], start=(blk == 0),
                                        stop=(blk == NB - 1))
                                nc.scalar.activation(g1[:, hc, :], ph, AF.Gelu,
                                                     bias=fc1b_t[:, hc, 0:1],
                                                     scale=1.0)
                        # fc2 + bias -> delta (residual added on host)
                        ot = xop.tile([128, 4, D], F32, name="ot")
                        for npass, (d0, d1) in enumerate([(0, 512), (512, 768)]):
                            nw = d1 - d0
                            po = ps_o.tile([128, 4, 512], F32, name="po")
                            for k in range(24):
                                f2t = w2s.tile([128, 512], F32R, name="f2t")
                                nc.gpsimd.dma_start(
                                    f2t[:, 0:nw],
                                    fc2wc[k * 128:(k + 1) * 128, d0:d1])
                                for m in range(4):
                                    nc.tensor.matmul(
                                        po[:, m, 0:nw],
                                        g1[:, k, m * 128:(m + 1) * 128],
                                        f2t[:, 0:nw],
                                        start=(k == 0), stop=(k == 23))
                            for m in range(4):
                                nc.vector.tensor_add(ot[:, m, d0:d1],
                                                     po[:, m, 0:nw],
                                                     fc2bB[:, d0:d1])
                        # int8 quant: per-partition absmax over 4x768 values
                        rmax = statp.tile([128, 1], F32, name="rmax")
                        nc.vector.tensor_reduce(
                            rmax, ot.rearrange("p m d -> p (m d)"),
                            axis=mybir.AxisListType.X, op=mybir.AluOpType.max,
                            apply_absolute_value=True)
                        nc.vector.tensor_scalar_max(rmax, rmax, 1e-30)
                        nc.sync.dma_start(out_s[g * 128:(g + 1) * 128, :], rmax)
                        qinv = statp.tile([128, 1], F32, name="qinv")
                        nc.vector.reciprocal(qinv, rmax)
                        nc.vector.tensor_scalar_mul(qinv, qinv, 127.0)
                        qt = xop.tile([128, 4, D], I8, name="qt")
                        nc.vector.tensor_scalar_mul(qt, ot, qinv)
                        nc.sync.dma_start(
                            out[g * TG:(g + 1) * TG, :]
                            .rearrange("(m p) d -> p m d", p=128), qt)
        return out, out_s

    return bass_shard_map(fused, mesh=_get_mesh(),
                          in_specs=(P(None, "core"), P("core")),
                          out_specs=(P("core"), P("core")))


# ---------------------------------------------------------------- host glue
def _build_consts(inp):
    M = build_mats()
    fc1w = np.ascontiguousarray(inp["fc1_w"], np.float32)
    fc2w = np.ascontiguousarray(inp["fc2_w"], np.float32)
    fc1b = np.ascontiguousarray(inp["fc1_b"], np.float32)
    fc2b = np.ascontiguousarray(inp["fc2_b"], np.float32)
    n2g = np.ascontiguousarray(inp["norm2_g"], np.float32)
    n2b = np.ascontiguousarray(inp["norm2_b"], np.float32)
    return dict(
        M,
        fc1wt=np.ascontiguousarray(
            fc1w.reshape(NB, BS, HID).transpose(1, 0, 2)),      # [96,8,3072]
        fc2w=fc2w,                                              # [3072,768]
        fc1bt=np.ascontiguousarray(
            fc1b.reshape(24, 128).T.reshape(128, 24, 1)),       # [128,24,1]
        fc2b=fc2b.reshape(1, D),                                # [1,768]
        n2gt=np.ascontiguousarray(
            n2g.reshape(NB, BS).T.reshape(BS, NB, 1)),          # [96,8,1]
        n2bt=np.ascontiguousarray(
            n2b.reshape(NB, BS).T.reshape(BS, NB, 1)),          # [96,8,1]
    )


def _build_wp(inp):
    g = inp["norm1_g"].astype(np.float32)
    b = inp["norm1_b"].astype(np.float32)
    w1 = inp["w1"].astype(np.float32)
    w2 = inp["w2"].astype(np.float32)
    b1 = inp["b1"].astype(np.float32)
    b2 = inp["b2"].astype(np.float32)
    wp = np.zeros((NCORES, BS, WPW), np.float32)
    for c in range(NCORES):
        blk = c
        gs = g[blk * BS:(blk + 1) * BS]
        wp[c, :, WCOL["w1r"]:WCOL["w1r"] + 96] = gs[:, None] * w1[0][blk]
        wp[c, :, WCOL["w1i"]:WCOL["w1i"] + 96] = gs[:, None] * w1[1][blk]
        wp[c, :, WCOL["w1in"]:WCOL["w1in"] + 96] = -(gs[:, None] * w1[1][blk])
        wp[c, :, WCOL["w2r"]:WCOL["w2r"] + 96] = w2[0][blk]
        wp[c, :, WCOL["w2i"]:WCOL["w2i"] + 96] = w2[1][blk]
        wp[c, :, WCOL["w2in"]:WCOL["w2in"] + 96] = -w2[1][blk]
        wp[c, :, BCOL["b1r"]] = b1[0][blk]
        wp[c, :, BCOL["b1i"]] = b1[1][blk]
        wp[c, :, BCOL["b2r"]] = b2[0][blk] - LAM
        wp[c, :, BCOL["b2i"]] = b2[1][blk] - LAM
        wp[c, :, BCOL["bdc"]] = b[blk * BS:(blk + 1) * BS] * SQHW
    return wp.reshape(NCORES * BS, WPW)


_cpu = None


def _get_cpu():
    global _cpu
    if _cpu is None:
        _cpu = jax.devices('cpu')[0]
    return _cpu


@jax.jit
def _dequant_jit(q, s, x):
    import jax.numpy as jnp
    # q: [HW, D] int8, s: [NCORES*NG*128] f32, x: [HW, D] f32
    sf = jnp.broadcast_to(
        s.reshape(NCORES, NG, 1, 128) * (1.0 / 127.0),
        (NCORES, NG, 4, 128)).reshape(HW, 1)
    return x + q.astype(jnp.float32) * sf


def _quant_rows(x2d, outq, tmp):
    """Per-row absmax int8 quantization into outq (no scales kept:
    LN1 is invariant to per-token scale)."""
    am = np.maximum(x2d.max(axis=1, keepdims=True),
                    -x2d.min(axis=1, keepdims=True))
    np.maximum(am, 1e-30, out=am)
    np.divide(127.0, am, out=am)
    np.multiply(x2d, am, out=tmp)
    np.rint(tmp, out=tmp)
    outq[:] = tmp  # integral-valued, cast is exact


def _dequant_rows(q, s, x):
    with jax.default_device(_get_cpu()):
        return np.asarray(_dequant_jit(q, s.reshape(-1), x))


_bufs = {}


def kernel(**inputs):
    inp = {k: np.asarray(v) for k, v in inputs.items()}
    consts = _build_consts(inp)
    key = hashlib.sha1(
        b"".join(np.ascontiguousarray(consts[k]).tobytes()
                 for k in sorted(consts))).hexdigest()
    if key not in _programs:
        _programs[key] = (make_fused(consts, 0), make_fused(consts, 1))
    f0, f1 = _programs[key]

    wpg = _build_wp(inp)
    wkey = hashlib.sha1(wpg.tobytes()).hexdigest()
    if wkey not in _wp_cache:
        _wp_cache.clear()
        _wp_cache[wkey] = jax.device_put(
            wpg, NamedSharding(_get_mesh(), P("core")))
    wp_dev = _wp_cache[wkey]

    if not _bufs:
        _bufs["xq"] = np.empty((2, HW, D), np.int8)
        _bufs["tmp"] = np.empty((2 * HW, D), np.float32)
    xq, tmp = _bufs["xq"], _bufs["tmp"]

    x32 = np.ascontiguousarray(inp["x"], np.float32).reshape(2, HW, D)
    _quant_rows(x32.reshape(2 * HW, D), xq.reshape(2 * HW, D), tmp)
    # one H2D put for both batches (fixed per-put cost dominates small puts)
    xq_dev = jax.device_put(xq, NamedSharding(_get_mesh(), P(None, "core")))
    # two pipelined launches of one batch each: batch 1 executes while
    # batch 0's delta streams back over the tunnel
    q0, s0 = f0(xq_dev, wp_dev)
    q1, s1 = f1(xq_dev, wp_dev)
    try:
        for a in (q0, s0, q1, s1):
            a.copy_to_host_async()
    except Exception:
        pass

    out = np.empty((2, HW, D), np.float32)
    for bi, (q, s) in enumerate(((q0, s0), (q1, s1))):
        qh = np.asarray(q)                       # [32768, 768] int8
        sh = np.asarray(s)                       # [NCORES*NG*128, 1]
        out[bi] = _dequant_rows(qh, sh, x32[bi])
    return out


if __name__ == "__main__":
    print("kernel module ok")
